# revision 15
# baseline (speedup 1.0000x reference)
"""BiMamba (2-direction Mamba-1 SSM) Trainium2 kernel, 8 NeuronCores.

Sharding: direction (2) x d_inner shard (4 x 256 channels). Each core computes
its direction's projections for its 256 channels, the full selective scan for
those channels (tensor_tensor_scan over time, one lane per (channel, state)),
and a partial out-projection. x_dbl partials are AllReduced across each
direction's 4 cores; the host sums out-proj partials, concatenates delta
shards, flips the reverse direction, and averages directions.

v2: causal conv folded into the in-projection matmuls (per-tap scaled weight
copies accumulating into PSUM with shifted moving operands); B/C row
broadcasts via stride-0-partition DMA into SBUF bf16 (enables 2x-rate bf16
DVE multiplies and frees PE/PSUM); scans run the full per-batch length (no
carry chaining); gate/out-proj path in bf16.
"""
import numpy as np
from contextlib import ExitStack

import ml_dtypes
import concourse.bass as bass
import concourse.mybir as mybir
import concourse.tile as tile
from concourse import bacc
from concourse.bass_utils import run_bass_kernel_spmd

F32 = mybir.dt.float32
F32R = mybir.dt.float32r
BF16 = mybir.dt.bfloat16
AF = mybir.ActivationFunctionType
ALU = mybir.AluOpType

DM = 512        # d_model
DI = 1024       # d_inner
DS = 16         # d_state
DC = 4          # d_conv
RK = 32         # dt_rank
RXP = 80        # padded x_dbl rows: dt_lr 0:32, B 32:48, pad, C 64:80
B = 2
L = 2048
BL = B * L      # 4096 tokens, batch-major
SH = 256        # channels per core
P = 128
N_CORES = 8
GP_NS = ()  # gpsimd cannot run tensor_tensor_scan (engine check)

_NC = None
_LAST_IN_MAPS = None


def _build():
    nc = bacc.Bacc("TRN2", target_bir_lowering=False, debug=False,
                   num_devices=N_CORES)
    din = lambda n, s, d=F32: nc.declare_dram_parameter(n, list(s), d,
                                                        isOutput=False)
    dout = lambda n, s: nc.declare_dram_parameter(n, list(s), F32, isOutput=True)

    xT = din("xT", (DM, BL))
    wu4 = din("wu4", (DM, DC * SH))     # per-tap conv-scaled in_proj (u half)
    wzT = din("wzT", (DM, SH))
    cb = din("cb", (P, 2))
    xpT = din("xpT", (SH, RXP), BF16)
    dwT = din("dwT", (RK, SH))
    dbc = din("dbc", (P, 2))
    Ac = din("Ac", (P, 2 * DS))
    Dc = din("Dc", (P, 2))
    opT = din("opT", (SH, DM), BF16)
    idn = din("idn", (P, P), BF16)
    z3 = din("z3", (P, 4))

    d_out = dout("d_out", (SH, BL))
    lr_out = dout("lr_out", (RK, BL))
    y_out = dout("y_out", (DM, BL))

    with tile.TileContext(nc) as tc, ExitStack() as ctx:
        wp = ctx.enter_context(tc.tile_pool(name="weights", bufs=1))
        big = ctx.enter_context(tc.tile_pool(name="big", bufs=1))
        dramp = ctx.enter_context(tc.tile_pool(name="dram", bufs=1, space="DRAM"))

        xp_sb = wp.tile([P, 2 * RXP], BF16)
        op_sb = wp.tile([P, 2 * DM], BF16)
        for kc in range(2):
            nc.sync.dma_start(xp_sb[:, kc * RXP:(kc + 1) * RXP],
                              xpT[kc * P:(kc + 1) * P, :])
            nc.sync.dma_start(op_sb[:, kc * DM:(kc + 1) * DM],
                              opT[kc * P:(kc + 1) * P, :])
        dw_sb = wp.tile([RK, SH], F32R)
        nc.sync.dma_start(dw_sb[:], dwT[:].bitcast(F32R))
        cb_sb = wp.tile([P, 2], F32)
        db_sb = wp.tile([P, 2], F32)
        A_sb = wp.tile([P, 2 * DS], F32)
        D_sb = wp.tile([P, 2], F32)
        for t_, src in ((cb_sb, cb), (db_sb, dbc), (A_sb, Ac), (D_sb, Dc)):
            nc.sync.dma_start(t_[:], src[:])
        ident = wp.tile([P, P], BF16)
        nc.sync.dma_start(ident[:], idn[:])

        # long-lived activations
        u2 = [big.tile([P, B, L], BF16, name=f"u{c}") for c in range(2)]
        z2 = [big.tile([P, B, L], BF16, name=f"z{c}") for c in range(2)]
        d2 = [big.tile([P, B, L], F32, name=f"d{c}") for c in range(2)]
        yg2 = [big.tile([P, B, L], BF16, name=f"yg{c}") for c in range(2)]
        xd_sb = big.tile([RXP, BL], F32R)

        # ---- phase B: in-projections with conv folded into the u-matmuls ----
        with nc.named_scope("inproj"), \
                tc.tile_pool(name="wu4p", bufs=1) as wu4p, \
                tc.tile_pool(name="xk", bufs=2) as xkp, \
                tc.tile_pool(name="psB", bufs=4, space="PSUM") as psB:
            wu_sb = wu4p.tile([P, 4 * DC * SH], F32R)  # [:, k,(j,ch)]
            wz_sb = wu4p.tile([P, 4 * SH], F32R)
            for k in range(4):
                nc.sync.dma_start(wu_sb[:, k * DC * SH:(k + 1) * DC * SH],
                                  wu4[k * P:(k + 1) * P, :].bitcast(F32R))
                nc.sync.dma_start(wz_sb[:, k * SH:(k + 1) * SH],
                                  wzT[k * P:(k + 1) * P, :].bitcast(F32R))
            for b in range(B):
                for tt in range(4):
                    tok = b * L + tt * 512
                    xks = []
                    for k in range(4):
                        xk_t = xkp.tile([P, 515], F32R, name=f"xk{k}",
                                        tag=f"xk{k}")
                        if tt == 0:
                            nc.sync.dma_start(xk_t[:, 0:3],
                                              z3[:, 0:3].bitcast(F32R))
                            nc.sync.dma_start(
                                xk_t[:, 3:515],
                                xT[k * P:(k + 1) * P, tok:tok + 512]
                                .bitcast(F32R))
                        else:
                            nc.sync.dma_start(
                                xk_t[:, 0:515],
                                xT[k * P:(k + 1) * P, tok - 3:tok + 512]
                                .bitcast(F32R))
                        xks.append(xk_t)
                    for c in range(2):
                        ps_u = psB.tile([P, 512], F32, tag="psu")
                        first = True
                        for k in range(4):
                            for j in (3, 2, 1, 0):
                                w_sl = wu_sb[:, k * DC * SH + j * SH + c * P:
                                             k * DC * SH + j * SH + (c + 1) * P]
                                nc.tensor.matmul(
                                    ps_u[:], w_sl, xks[k][:, j:j + 512],
                                    start=first, stop=(k == 3 and j == 0))
                                first = False
                        nc.scalar.activation(u2[c][:, b, tt * 512:(tt + 1) * 512],
                                             ps_u[:], AF.Silu,
                                             bias=cb_sb[:, c:c + 1])
                        ps_z = psB.tile([P, 512], F32, tag="psz")
                        for k in range(4):
                            nc.tensor.matmul(
                                ps_z[:], wz_sb[:, k * SH + c * P:k * SH + (c + 1) * P],
                                xks[k][:, 3:515],
                                start=(k == 0), stop=(k == 3))
                        nc.scalar.activation(z2[c][:, b, tt * 512:(tt + 1) * 512],
                                             ps_z[:], AF.Silu)

        # ---- phase D: x_dbl partials -> per-half-batch AllReduces ----
        with nc.named_scope("xdbl"), \
                tc.tile_pool(name="psD", bufs=2, space="PSUM") as psD, \
                tc.tile_pool(name="xds", bufs=2) as xds:
            B_dram = dramp.tile([DS, BL], BF16, name="B_dram")
            C_dram = dramp.tile([DS, BL], BF16, name="C_dram")
            for b in range(B):
                for hf in range(2):
                    base = b * L + hf * 1024
                    xd_in = dramp.tile([RXP, 1024], F32, name=f"xdi{b}{hf}",
                                       tag=f"xdi{b}{hf}")
                    xd_red = dramp.tile([RXP, 1024], F32, name=f"xdr{b}{hf}",
                                        tag=f"xdr{b}{hf}")
                    for t2 in range(2):
                        tok = base + t2 * 512
                        ps = psD.tile([RXP, 512], F32, tag="psd")
                        for kc in range(2):
                            nc.tensor.matmul(
                                ps[:], xp_sb[:, kc * RXP:(kc + 1) * RXP],
                                u2[kc][:, b, tok - b * L:tok - b * L + 512],
                                start=(kc == 0), stop=(kc == 1))
                        xo = xds.tile([RXP, 512], F32, tag="xo")
                        nc.scalar.copy(xo[:], ps[:])
                        nc.sync.dma_start(xd_in[:, t2 * 512:(t2 + 1) * 512],
                                          xo[:])
                    nc.gpsimd.collective_compute(
                        "AllReduce", ALU.add,
                        replica_groups=[[0, 1, 2, 3], [4, 5, 6, 7]],
                        ins=[xd_in.opt()], outs=[xd_red.opt()])
                    hsl = slice(base, base + 1024)
                    nc.sync.dma_start(xd_sb[:, hsl], xd_red[:].bitcast(F32R))
                    nc.sync.dma_start(lr_out[:, hsl],
                                      xd_sb[0:RK, hsl].bitcast(F32))
                    B_bf = xds.tile([48, 1024], BF16, name="B_bf", tag="B_bf")
                    C_bf = xds.tile([RXP, 1024], BF16, name="C_bf", tag="C_bf")
                    nc.scalar.copy(B_bf[32:48, :], xd_sb[32:48, hsl].bitcast(F32))
                    nc.scalar.copy(C_bf[64:80, :], xd_sb[64:80, hsl].bitcast(F32))
                    nc.sync.dma_start(B_dram[:, hsl], B_bf[32:48, :])
                    nc.sync.dma_start(C_dram[:, hsl], C_bf[64:80, :])

        # ---- phase E: delta = softplus(dt_lr @ dt_w.T + dt_b) ----
        with nc.named_scope("delta"), \
                tc.tile_pool(name="psE", bufs=2, space="PSUM") as psE, \
                tc.tile_pool(name="spt", bufs=2) as spt:
            for b in range(B):
                for tt in range(4):
                    tok = b * L + tt * 512
                    for c in range(2):
                        ps = psE.tile([P, 512], F32, tag="pse")
                        nc.tensor.matmul(ps[:], dw_sb[:, c * P:(c + 1) * P],
                                         xd_sb[0:RK, tok:tok + 512],
                                         start=True, stop=True)
                        ax = spt.tile([P, 512], F32, tag="ax")
                        nc.scalar.activation(ax[:], ps[:], AF.Abs,
                                             bias=db_sb[:, c:c + 1])
                        ex = spt.tile([P, 512], F32, tag="ex")
                        nc.scalar.activation(ex[:], ax[:], AF.Exp, scale=-1.0)
                        ln = spt.tile([P, 512], F32, tag="ln")
                        nc.scalar.activation(ln[:], ex[:], AF.Ln, bias=1.0)
                        t1 = spt.tile([P, 512], F32, tag="t1")
                        nc.vector.tensor_scalar_add(t1[:], ps[:],
                                                    db_sb[:, c:c + 1])
                        nc.vector.scalar_tensor_tensor(
                            d2[c][:, b, tt * 512:(tt + 1) * 512], t1[:], 0.0,
                            ln[:], op0=ALU.max, op1=ALU.add)
            for c in range(2):
                nc.sync.dma_start(d_out[c * P:(c + 1) * P, :],
                                  d2[c][:].rearrange("p b l -> p (b l)"))

        # ---- phase G: selective scan (1024-chunks, carried state) ----
        hcar = big.tile([P, 2 * DS], F32)
        with nc.named_scope("scan"), \
                tc.tile_pool(name="psY", bufs=4, space="PSUM") as psY, \
                tc.tile_pool(name="sct", bufs=2) as sct:
            for b in range(B):
                for tcn in range(2):
                    TCH = 1024
                    bsl = slice(b * L + tcn * TCH, b * L + (tcn + 1) * TCH)
                    csl = slice(tcn * TCH, (tcn + 1) * TCH)
                    dus = []
                    for c in range(2):
                        du = sct.tile([P, TCH], BF16, name=f"du{c}",
                                      tag=f"du{c}", bufs=2)
                        nc.vector.tensor_mul(du[:], d2[c][:, b, csl],
                                             u2[c][:, b, csl])
                        dus.append(du)
                    y_ps = [psY.tile([P, TCH], F32, name=f"yps{i}", tag="y")
                            for i in range(2)]
                    for n in range(DS):
                        Bbc = sct.tile([P, TCH], BF16, tag="Bbc")
                        Cbc = sct.tile([P, TCH], BF16, tag="Cbc")
                        brow = B_dram[n:n + 1, bsl]
                        crow = C_dram[n:n + 1, bsl]
                        nc.scalar.dma_start(Bbc[:], bass.AP(
                            tensor=brow.tensor, offset=brow.offset,
                            ap=[[0, P]] + [list(p) for p in brow.ap[1:]]))
                        nc.scalar.dma_start(Cbc[:], bass.AP(
                            tensor=crow.tensor, offset=crow.offset,
                            ap=[[0, P]] + [list(p) for p in crow.ap[1:]]))
                        for c in range(2):
                            a_t = sct.tile([P, TCH], F32, tag="a")
                            nc.scalar.activation(
                                a_t[:], d2[c][:, b, csl], AF.Exp,
                                scale=A_sb[:, c * DS + n:c * DS + n + 1])
                            b_t = sct.tile([P, TCH], BF16, tag="bt", bufs=3)
                            nc.vector.tensor_mul(b_t[:], dus[c][:], Bbc[:])
                            h_t = sct.tile([P, TCH], BF16, tag="h", bufs=3)
                            slot = c * DS + n
                            init = 0.0 if tcn == 0 else hcar[:, slot:slot + 1]
                            nc.vector.tensor_tensor_scan(
                                h_t[:], a_t[:], b_t[:], init,
                                op0=ALU.mult, op1=ALU.add)
                            if tcn == 0:
                                nc.vector.tensor_copy(hcar[:, slot:slot + 1],
                                                      h_t[:, TCH - 1:TCH])
                            hc_t = sct.tile([P, TCH], BF16, tag="hc", bufs=3)
                            nc.vector.tensor_mul(hc_t[:], h_t[:], Cbc[:])
                            for q in range(2):
                                hs = slice(q * 512, (q + 1) * 512)
                                nc.tensor.matmul(y_ps[c][:, hs], ident[:],
                                                 hc_t[:, hs], start=(n == 0),
                                                 stop=(n == DS - 1))
                    for c in range(2):
                        y1 = sct.tile([P, TCH], F32, tag="a")
                        nc.vector.scalar_tensor_tensor(
                            y1[:], u2[c][:, b, csl], D_sb[:, c:c + 1],
                            y_ps[c][:], op0=ALU.mult, op1=ALU.add)
                        nc.vector.tensor_mul(yg2[c][:, b, csl], y1[:],
                                             z2[c][:, b, csl])

        # ---- phase H: out-projection partials ----
        with nc.named_scope("outproj"), \
                tc.tile_pool(name="psH", bufs=2, space="PSUM") as psH, \
                tc.tile_pool(name="osb", bufs=2) as osb:
            for b in range(B):
                for tt in range(4):
                    tok = b * L + tt * 512
                    for dmt in range(4):
                        ps = psH.tile([P, 512], F32, tag="psh")
                        for kc in range(2):
                            nc.tensor.matmul(
                                ps[:],
                                op_sb[:, kc * DM + dmt * P:kc * DM + (dmt + 1) * P],
                                yg2[kc][:, b, tt * 512:(tt + 1) * 512],
                                start=(kc == 0), stop=(kc == 1))
                        o = osb.tile([P, 512], F32, tag="o")
                        nc.scalar.copy(o[:], ps[:])
                        nc.sync.dma_start(
                            y_out[dmt * P:(dmt + 1) * P, tok:tok + 512], o[:])

    nc.compile()
    return nc


def _get_nc():
    global _NC
    if _NC is None:
        _NC = _build()
    return _NC


def kernel(**inputs):
    x = np.asarray(inputs["x"], np.float32)
    nc = _get_nc()

    idn = np.eye(P, dtype=np.float32).astype(ml_dtypes.bfloat16)

    in_maps = []
    for g, pfx in enumerate(("f_", "r_")):
        W = np.asarray(inputs[pfx + "in_proj"], np.float32)
        conv_w = np.asarray(inputs[pfx + "conv_w"], np.float32)
        conv_b = np.asarray(inputs[pfx + "conv_b"], np.float32)
        x_proj = np.asarray(inputs[pfx + "x_proj"], np.float32)
        dt_w = np.asarray(inputs[pfx + "dt_w"], np.float32)
        dt_b = np.asarray(inputs[pfx + "dt_b"], np.float32)
        A = -np.exp(np.asarray(inputs[pfx + "A_log"], np.float32))
        D_ = np.asarray(inputs[pfx + "D"], np.float32)
        out_proj = np.asarray(inputs[pfx + "out_proj"], np.float32)

        xg = x if g == 0 else x[:, ::-1]
        xT = np.ascontiguousarray(xg.transpose(2, 0, 1).reshape(DM, BL))
        for s in range(4):
            ch = slice(s * SH, (s + 1) * SH)
            cm = lambda a: np.ascontiguousarray(a.astype(np.float32))
            # wu4[dm, j, ch] = W_u[ch, dm] * conv_w[ch, j]
            wu4 = (W[:DI][ch].T[:, None, :] *
                   conv_w[ch].T[None, :, :]).reshape(DM, DC * SH)
            xp_pad = np.zeros((SH, RXP), np.float32)
            xp_pad[:, 0:RK + DS] = x_proj[0:RK + DS, ch].T
            xp_pad[:, 64:80] = x_proj[RK + DS:RK + 2 * DS, ch].T
            in_maps.append({
                "xT": xT,
                "wu4": cm(wu4),
                "wzT": cm(W[DI:][ch].T),
                "cb": cm(conv_b[ch].reshape(2, P).T),
                "xpT": np.ascontiguousarray(xp_pad.astype(ml_dtypes.bfloat16)),
                "dwT": cm(dt_w[ch].T),
                "dbc": cm(dt_b[ch].reshape(2, P).T),
                "Ac": cm(A[ch].reshape(2, P, DS).transpose(1, 0, 2)
                         .reshape(P, 2 * DS)),
                "Dc": cm(D_[ch].reshape(2, P).T),
                "opT": np.ascontiguousarray(
                    out_proj[:, ch].T.astype(ml_dtypes.bfloat16)),
                "idn": idn,
                "z3": np.zeros((P, 4), np.float32),
            })

    global _LAST_IN_MAPS
    _LAST_IN_MAPS = in_maps
    res = run_bass_kernel_spmd(nc, in_maps, list(range(N_CORES))).results

    outs, dts, dts_lr = [], [], []
    for g in range(2):
        delta = np.concatenate([res[g * 4 + s]["d_out"] for s in range(4)],
                               axis=0)  # (DI, BL)
        dts.append(delta.reshape(DI, B, L).transpose(1, 2, 0))
        dts_lr.append(res[g * 4]["lr_out"].reshape(RK, B, L).transpose(1, 2, 0))
        y = np.sum([res[g * 4 + s]["y_out"] for s in range(4)], axis=0)
        y = y.reshape(DM, B, L).transpose(1, 2, 0)  # (B, L, DM)
        outs.append(y)
    outs[1] = outs[1][:, ::-1]
    out = (outs[0] + outs[1]) / 2
    return (out.astype(np.float32),
            np.stack(dts).astype(np.float32),
            np.stack(dts_lr).astype(np.float32))


# revision 16
# speedup vs baseline: 1.0725x; 1.0725x over previous
"""BiMamba (2-direction Mamba-1 SSM) Trainium2 kernel, 8 NeuronCores.

Sharding: direction (2) x d_inner shard (4 x 256 channels). Each core computes
its direction's projections for its 256 channels, the full selective scan for
those channels (tensor_tensor_scan over time, one lane per (channel, state)),
and a partial out-projection. x_dbl partials are AllReduced across each
direction's 4 cores; the host sums out-proj partials, concatenates delta
shards, flips the reverse direction, and averages directions.

v2: causal conv folded into the in-projection matmuls (per-tap scaled weight
copies accumulating into PSUM with shifted moving operands); B/C row
broadcasts via stride-0-partition DMA into SBUF bf16 (enables 2x-rate bf16
DVE multiplies and frees PE/PSUM); scans run the full per-batch length (no
carry chaining); gate/out-proj path in bf16.
"""
import numpy as np
from contextlib import ExitStack

import ml_dtypes
import concourse.bass as bass
import concourse.mybir as mybir
import concourse.tile as tile
from concourse import bacc
from concourse.bass_utils import run_bass_kernel_spmd

F32 = mybir.dt.float32
F32R = mybir.dt.float32r
BF16 = mybir.dt.bfloat16
AF = mybir.ActivationFunctionType
ALU = mybir.AluOpType

DM = 512        # d_model
DI = 1024       # d_inner
DS = 16         # d_state
DC = 4          # d_conv
RK = 32         # dt_rank
RXP = 80        # padded x_dbl rows: dt_lr 0:32, B 32:48, pad, C 64:80
B = 2
L = 2048
BL = B * L      # 4096 tokens, batch-major
SH = 256        # channels per core
P = 128
N_CORES = 8
GP_NS = ()  # gpsimd cannot run tensor_tensor_scan (engine check)

_NC = None
_LAST_IN_MAPS = None


def _build():
    nc = bacc.Bacc("TRN2", target_bir_lowering=False, debug=False,
                   num_devices=N_CORES)
    din = lambda n, s, d=F32: nc.declare_dram_parameter(n, list(s), d,
                                                        isOutput=False)
    dout = lambda n, s: nc.declare_dram_parameter(n, list(s), F32, isOutput=True)

    xT = din("xT", (DM, BL))
    wu4 = din("wu4", (DM, DC * SH))     # per-tap conv-scaled in_proj (u half)
    wzT = din("wzT", (DM, SH))
    cb = din("cb", (P, 2))
    xpT = din("xpT", (SH, RXP), BF16)
    dwT = din("dwT", (RK, SH))
    dbc = din("dbc", (P, 2))
    Ac = din("Ac", (P, 2 * DS))
    Dc = din("Dc", (P, 2))
    opT = din("opT", (SH, DM), BF16)
    idn = din("idn", (P, P), BF16)
    z3 = din("z3", (P, 4))

    d_out = dout("d_out", (SH, BL))
    lr_out = dout("lr_out", (RK, BL))
    y_out = dout("y_out", (DM, BL))

    with tile.TileContext(nc) as tc, ExitStack() as ctx:
        wp = ctx.enter_context(tc.tile_pool(name="weights", bufs=1))
        big = ctx.enter_context(tc.tile_pool(name="big", bufs=1))
        dramp = ctx.enter_context(tc.tile_pool(name="dram", bufs=1, space="DRAM"))

        xp_sb = wp.tile([P, 2 * RXP], BF16)
        op_sb = wp.tile([P, 2 * DM], BF16)
        for kc in range(2):
            nc.sync.dma_start(xp_sb[:, kc * RXP:(kc + 1) * RXP],
                              xpT[kc * P:(kc + 1) * P, :])
            nc.sync.dma_start(op_sb[:, kc * DM:(kc + 1) * DM],
                              opT[kc * P:(kc + 1) * P, :])
        dw_sb = wp.tile([RK, SH], F32R)
        nc.sync.dma_start(dw_sb[:], dwT[:].bitcast(F32R))
        cb_sb = wp.tile([P, 2], F32)
        db_sb = wp.tile([P, 2], F32)
        A_sb = wp.tile([P, 2 * DS], F32)
        D_sb = wp.tile([P, 2], F32)
        for t_, src in ((cb_sb, cb), (db_sb, dbc), (A_sb, Ac), (D_sb, Dc)):
            nc.sync.dma_start(t_[:], src[:])
        ident = wp.tile([P, P], BF16)
        nc.sync.dma_start(ident[:], idn[:])

        # long-lived activations
        u2 = [big.tile([P, B, L], BF16, name=f"u{c}") for c in range(2)]
        z2 = [big.tile([P, B, L], BF16, name=f"z{c}") for c in range(2)]
        d2 = [big.tile([P, B, L], F32, name=f"d{c}") for c in range(2)]
        yg2 = [big.tile([P, B, L], BF16, name=f"yg{c}") for c in range(2)]
        xd_sb = big.tile([RXP, BL], F32R)

        # ---- phase B: in-projections with conv folded into the u-matmuls ----
        with nc.named_scope("inproj"), \
                tc.tile_pool(name="wu4p", bufs=1) as wu4p, \
                tc.tile_pool(name="xk", bufs=2) as xkp, \
                tc.tile_pool(name="psB", bufs=4, space="PSUM") as psB:
            wu_sb = wu4p.tile([P, 4 * DC * SH], F32R)  # [:, k,(j,ch)]
            wz_sb = wu4p.tile([P, 4 * SH], F32R)
            for k in range(4):
                nc.sync.dma_start(wu_sb[:, k * DC * SH:(k + 1) * DC * SH],
                                  wu4[k * P:(k + 1) * P, :].bitcast(F32R))
                nc.sync.dma_start(wz_sb[:, k * SH:(k + 1) * SH],
                                  wzT[k * P:(k + 1) * P, :].bitcast(F32R))
            for b in range(B):
                for tt in range(4):
                    tok = b * L + tt * 512
                    xks = []
                    for k in range(4):
                        xk_t = xkp.tile([P, 515], F32R, name=f"xk{k}",
                                        tag=f"xk{k}")
                        if tt == 0:
                            nc.sync.dma_start(xk_t[:, 0:3],
                                              z3[:, 0:3].bitcast(F32R))
                            nc.sync.dma_start(
                                xk_t[:, 3:515],
                                xT[k * P:(k + 1) * P, tok:tok + 512]
                                .bitcast(F32R))
                        else:
                            nc.sync.dma_start(
                                xk_t[:, 0:515],
                                xT[k * P:(k + 1) * P, tok - 3:tok + 512]
                                .bitcast(F32R))
                        xks.append(xk_t)
                    for c in range(2):
                        ps_u = psB.tile([P, 512], F32, tag="psu")
                        first = True
                        for k in range(4):
                            for j in (3, 2, 1, 0):
                                w_sl = wu_sb[:, k * DC * SH + j * SH + c * P:
                                             k * DC * SH + j * SH + (c + 1) * P]
                                nc.tensor.matmul(
                                    ps_u[:], w_sl, xks[k][:, j:j + 512],
                                    start=first, stop=(k == 3 and j == 0))
                                first = False
                        nc.scalar.activation(u2[c][:, b, tt * 512:(tt + 1) * 512],
                                             ps_u[:], AF.Silu,
                                             bias=cb_sb[:, c:c + 1])
                        ps_z = psB.tile([P, 512], F32, tag="psz")
                        for k in range(4):
                            nc.tensor.matmul(
                                ps_z[:], wz_sb[:, k * SH + c * P:k * SH + (c + 1) * P],
                                xks[k][:, 3:515],
                                start=(k == 0), stop=(k == 3))
                        nc.scalar.activation(z2[c][:, b, tt * 512:(tt + 1) * 512],
                                             ps_z[:], AF.Silu)

        # ---- phase D: x_dbl partial -> per-batch AllReduce ----
        with nc.named_scope("xdbl"), \
                tc.tile_pool(name="psD", bufs=2, space="PSUM") as psD, \
                tc.tile_pool(name="xds", bufs=2) as xds:
            B_dram = dramp.tile([DS, BL], BF16, name="B_dram")
            C_dram = dramp.tile([DS, BL], BF16, name="C_dram")
            for b in range(B):
                xd_in = dramp.tile([RXP, L], F32, name=f"xdi{b}")
                xd_red = dramp.tile([RXP, L], F32, name=f"xdr{b}")
                for tt in range(4):
                    tok = b * L + tt * 512
                    ps = psD.tile([RXP, 512], F32, tag="psd")
                    for kc in range(2):
                        nc.tensor.matmul(
                            ps[:], xp_sb[:, kc * RXP:(kc + 1) * RXP],
                            u2[kc][:, b, tt * 512:(tt + 1) * 512],
                            start=(kc == 0), stop=(kc == 1))
                    xo = xds.tile([RXP, 512], F32, tag="xo")
                    nc.scalar.copy(xo[:], ps[:])
                    nc.sync.dma_start(xd_in[:, tt * 512:(tt + 1) * 512], xo[:])
                nc.gpsimd.collective_compute(
                    "AllReduce", ALU.add,
                    replica_groups=[[0, 1, 2, 3], [4, 5, 6, 7]],
                    ins=[xd_in.opt()], outs=[xd_red.opt()])
                bsl = slice(b * L, (b + 1) * L)
                nc.sync.dma_start(xd_sb[:, bsl], xd_red[:].bitcast(F32R))
                nc.sync.dma_start(lr_out[:, bsl], xd_sb[0:RK, bsl].bitcast(F32))
                B_bf = xds.tile([48, L], BF16, name="B_bf", tag="B_bf")
                C_bf = xds.tile([RXP, L], BF16, name="C_bf", tag="C_bf")
                nc.scalar.copy(B_bf[32:48, :], xd_sb[32:48, bsl].bitcast(F32))
                nc.scalar.copy(C_bf[64:80, :], xd_sb[64:80, bsl].bitcast(F32))
                nc.sync.dma_start(B_dram[:, bsl], B_bf[32:48, :])
                nc.sync.dma_start(C_dram[:, bsl], C_bf[64:80, :])

        # ---- phase E: delta = softplus(dt_lr @ dt_w.T + dt_b) ----
        with nc.named_scope("delta"), \
                tc.tile_pool(name="psE", bufs=2, space="PSUM") as psE, \
                tc.tile_pool(name="spt", bufs=2) as spt:
            for b in range(B):
                for tt in range(4):
                    tok = b * L + tt * 512
                    for c in range(2):
                        ps = psE.tile([P, 512], F32, tag="pse")
                        nc.tensor.matmul(ps[:], dw_sb[:, c * P:(c + 1) * P],
                                         xd_sb[0:RK, tok:tok + 512],
                                         start=True, stop=True)
                        ax = spt.tile([P, 512], F32, tag="ax")
                        nc.scalar.activation(ax[:], ps[:], AF.Abs,
                                             bias=db_sb[:, c:c + 1])
                        ex = spt.tile([P, 512], F32, tag="ex")
                        nc.scalar.activation(ex[:], ax[:], AF.Exp, scale=-1.0)
                        ln = spt.tile([P, 512], F32, tag="ln")
                        nc.scalar.activation(ln[:], ex[:], AF.Ln, bias=1.0)
                        t1 = spt.tile([P, 512], F32, tag="t1")
                        nc.vector.tensor_scalar_add(t1[:], ps[:],
                                                    db_sb[:, c:c + 1])
                        nc.vector.scalar_tensor_tensor(
                            d2[c][:, b, tt * 512:(tt + 1) * 512], t1[:], 0.0,
                            ln[:], op0=ALU.max, op1=ALU.add)
            for c in range(2):
                nc.sync.dma_start(d_out[c * P:(c + 1) * P, :],
                                  d2[c][:].rearrange("p b l -> p (b l)"))

        # ---- phase G: selective scan (full per-batch length, no carry) ----
        with nc.named_scope("scan"), \
                tc.tile_pool(name="psY", bufs=2, space="PSUM") as psY, \
                tc.tile_pool(name="sct", bufs=2) as sct:
            for b in range(B):
                bsl = slice(b * L, (b + 1) * L)
                dus = []
                for c in range(2):
                    du = sct.tile([P, L], BF16, name=f"du{c}", tag=f"du{c}",
                                  bufs=1)
                    nc.vector.tensor_mul(du[:], d2[c][:, b, :], u2[c][:, b, :])
                    dus.append(du)
                y_ps = [psY.tile([P, L], F32, name=f"yps{i}", tag="y")
                        for i in range(2)]
                for n in range(DS):
                    Bbc = sct.tile([P, L], BF16, tag="Bbc")
                    Cbc = sct.tile([P, L], BF16, tag="Cbc")
                    brow = B_dram[n:n + 1, bsl]
                    crow = C_dram[n:n + 1, bsl]
                    nc.scalar.dma_start(Bbc[:], bass.AP(
                        tensor=brow.tensor, offset=brow.offset,
                        ap=[[0, P]] + [list(p) for p in brow.ap[1:]]))
                    nc.scalar.dma_start(Cbc[:], bass.AP(
                        tensor=crow.tensor, offset=crow.offset,
                        ap=[[0, P]] + [list(p) for p in crow.ap[1:]]))
                    for c in range(2):
                        a_t = sct.tile([P, L], F32, tag="a")
                        nc.scalar.activation(
                            a_t[:], d2[c][:, b, :], AF.Exp,
                            scale=A_sb[:, c * DS + n:c * DS + n + 1])
                        b_t = sct.tile([P, L], BF16, tag="bt", bufs=3)
                        nc.vector.tensor_mul(b_t[:], dus[c][:], Bbc[:])
                        h_t = sct.tile([P, L], BF16, tag="h", bufs=3)
                        eng = nc.gpsimd if n in GP_NS else nc.vector
                        eng.tensor_tensor_scan(h_t[:], a_t[:], b_t[:], 0.0,
                                               op0=ALU.mult, op1=ALU.add)
                        hc_t = sct.tile([P, L], BF16, tag="hc", bufs=3)
                        nc.vector.tensor_mul(hc_t[:], h_t[:], Cbc[:])
                        for q in range(4):
                            hs = slice(q * 512, (q + 1) * 512)
                            nc.tensor.matmul(y_ps[c][:, hs], ident[:],
                                             hc_t[:, hs], start=(n == 0),
                                             stop=(n == DS - 1))
                for c in range(2):
                    y1 = sct.tile([P, L], F32, tag="a")
                    nc.vector.scalar_tensor_tensor(
                        y1[:], u2[c][:, b, :], D_sb[:, c:c + 1],
                        y_ps[c][:], op0=ALU.mult, op1=ALU.add)
                    nc.vector.tensor_mul(yg2[c][:, b, :], y1[:], z2[c][:, b, :])

        # ---- phase H: out-projection partials ----
        with nc.named_scope("outproj"), \
                tc.tile_pool(name="psH", bufs=2, space="PSUM") as psH, \
                tc.tile_pool(name="osb", bufs=2) as osb:
            for b in range(B):
                for tt in range(4):
                    tok = b * L + tt * 512
                    for dmt in range(4):
                        ps = psH.tile([P, 512], F32, tag="psh")
                        for kc in range(2):
                            nc.tensor.matmul(
                                ps[:],
                                op_sb[:, kc * DM + dmt * P:kc * DM + (dmt + 1) * P],
                                yg2[kc][:, b, tt * 512:(tt + 1) * 512],
                                start=(kc == 0), stop=(kc == 1))
                        o = osb.tile([P, 512], F32, tag="o")
                        nc.scalar.copy(o[:], ps[:])
                        nc.sync.dma_start(
                            y_out[dmt * P:(dmt + 1) * P, tok:tok + 512], o[:])

    nc.compile()
    return nc


def _get_nc():
    global _NC
    if _NC is None:
        _NC = _build()
    return _NC


def kernel(**inputs):
    x = np.asarray(inputs["x"], np.float32)
    nc = _get_nc()

    idn = np.eye(P, dtype=np.float32).astype(ml_dtypes.bfloat16)

    in_maps = []
    for g, pfx in enumerate(("f_", "r_")):
        W = np.asarray(inputs[pfx + "in_proj"], np.float32)
        conv_w = np.asarray(inputs[pfx + "conv_w"], np.float32)
        conv_b = np.asarray(inputs[pfx + "conv_b"], np.float32)
        x_proj = np.asarray(inputs[pfx + "x_proj"], np.float32)
        dt_w = np.asarray(inputs[pfx + "dt_w"], np.float32)
        dt_b = np.asarray(inputs[pfx + "dt_b"], np.float32)
        A = -np.exp(np.asarray(inputs[pfx + "A_log"], np.float32))
        D_ = np.asarray(inputs[pfx + "D"], np.float32)
        out_proj = np.asarray(inputs[pfx + "out_proj"], np.float32)

        xg = x if g == 0 else x[:, ::-1]
        xT = np.ascontiguousarray(xg.transpose(2, 0, 1).reshape(DM, BL))
        for s in range(4):
            ch = slice(s * SH, (s + 1) * SH)
            cm = lambda a: np.ascontiguousarray(a.astype(np.float32))
            # wu4[dm, j, ch] = W_u[ch, dm] * conv_w[ch, j]
            wu4 = (W[:DI][ch].T[:, None, :] *
                   conv_w[ch].T[None, :, :]).reshape(DM, DC * SH)
            xp_pad = np.zeros((SH, RXP), np.float32)
            xp_pad[:, 0:RK + DS] = x_proj[0:RK + DS, ch].T
            xp_pad[:, 64:80] = x_proj[RK + DS:RK + 2 * DS, ch].T
            in_maps.append({
                "xT": xT,
                "wu4": cm(wu4),
                "wzT": cm(W[DI:][ch].T),
                "cb": cm(conv_b[ch].reshape(2, P).T),
                "xpT": np.ascontiguousarray(xp_pad.astype(ml_dtypes.bfloat16)),
                "dwT": cm(dt_w[ch].T),
                "dbc": cm(dt_b[ch].reshape(2, P).T),
                "Ac": cm(A[ch].reshape(2, P, DS).transpose(1, 0, 2)
                         .reshape(P, 2 * DS)),
                "Dc": cm(D_[ch].reshape(2, P).T),
                "opT": np.ascontiguousarray(
                    out_proj[:, ch].T.astype(ml_dtypes.bfloat16)),
                "idn": idn,
                "z3": np.zeros((P, 4), np.float32),
            })

    global _LAST_IN_MAPS
    _LAST_IN_MAPS = in_maps
    res = run_bass_kernel_spmd(nc, in_maps, list(range(N_CORES))).results

    outs, dts, dts_lr = [], [], []
    for g in range(2):
        delta = np.concatenate([res[g * 4 + s]["d_out"] for s in range(4)],
                               axis=0)  # (DI, BL)
        dts.append(delta.reshape(DI, B, L).transpose(1, 2, 0))
        dts_lr.append(res[g * 4]["lr_out"].reshape(RK, B, L).transpose(1, 2, 0))
        y = np.sum([res[g * 4 + s]["y_out"] for s in range(4)], axis=0)
        y = y.reshape(DM, B, L).transpose(1, 2, 0)  # (B, L, DM)
        outs.append(y)
    outs[1] = outs[1][:, ::-1]
    out = (outs[0] + outs[1]) / 2
    return (out.astype(np.float32),
            np.stack(dts).astype(np.float32),
            np.stack(dts_lr).astype(np.float32))


# revision 18
# speedup vs baseline: 1.1235x; 1.0476x over previous
"""BiMamba (2-direction Mamba-1 SSM) Trainium2 kernel, 8 NeuronCores.

Sharding: direction (2) x d_inner shard (4 x 256 channels). Each core computes
its direction's projections for its 256 channels, the full selective scan for
those channels (tensor_tensor_scan over time, one lane per (channel, state)),
and a partial out-projection. x_dbl partials are AllReduced across each
direction's 4 cores; the host sums out-proj partials, concatenates delta
shards, flips the reverse direction, and averages directions.

v2: causal conv folded into the in-projection matmuls (per-tap scaled weight
copies accumulating into PSUM with shifted moving operands); B/C row
broadcasts via stride-0-partition DMA into SBUF bf16 (enables 2x-rate bf16
DVE multiplies and frees PE/PSUM); scans run the full per-batch length (no
carry chaining); gate/out-proj path in bf16.
"""
import numpy as np
from contextlib import ExitStack

import ml_dtypes
import concourse.bass as bass
import concourse.mybir as mybir
import concourse.tile as tile
from concourse import bacc
from concourse.bass_utils import run_bass_kernel_spmd

F32 = mybir.dt.float32
F32R = mybir.dt.float32r
BF16 = mybir.dt.bfloat16
AF = mybir.ActivationFunctionType
ALU = mybir.AluOpType

DM = 512        # d_model
DI = 1024       # d_inner
DS = 16         # d_state
DC = 4          # d_conv
RK = 32         # dt_rank
RXP = 80        # padded x_dbl rows: dt_lr 0:32, B 32:48, pad, C 64:80
B = 2
L = 2048
BL = B * L      # 4096 tokens, batch-major
SH = 256        # channels per core
P = 128
N_CORES = 8
GP_NS = ()  # gpsimd cannot run tensor_tensor_scan (engine check)

_NC = None
_LAST_IN_MAPS = None


def _build():
    nc = bacc.Bacc("TRN2", target_bir_lowering=False, debug=False,
                   num_devices=N_CORES)
    din = lambda n, s, d=F32: nc.declare_dram_parameter(n, list(s), d,
                                                        isOutput=False)
    dout = lambda n, s: nc.declare_dram_parameter(n, list(s), F32, isOutput=True)

    xT = din("xT", (DM, BL))
    wu4 = din("wu4", (DM, DC * SH))     # per-tap conv-scaled in_proj (u half)
    wzT = din("wzT", (DM, SH))
    cb = din("cb", (P, 2))
    xpT = din("xpT", (SH, RXP), BF16)
    dwT = din("dwT", (RK, SH))
    dbc = din("dbc", (P, 2))
    Ac = din("Ac", (P, 2 * DS))
    Dc = din("Dc", (P, 2))
    opT = din("opT", (SH, DM), BF16)
    idn = din("idn", (P, P), BF16)
    z3 = din("z3", (P, 4))

    d_out = dout("d_out", (SH, BL))
    lr_out = dout("lr_out", (RK, BL))
    y_out = dout("y_out", (DM, BL))

    with tile.TileContext(nc) as tc, ExitStack() as ctx:
        wp = ctx.enter_context(tc.tile_pool(name="weights", bufs=1))
        big = ctx.enter_context(tc.tile_pool(name="big", bufs=1))
        dramp = ctx.enter_context(tc.tile_pool(name="dram", bufs=1, space="DRAM"))

        xp_sb = wp.tile([P, 2 * RXP], BF16)
        op_sb = wp.tile([P, 2 * DM], BF16)
        for kc in range(2):
            nc.sync.dma_start(xp_sb[:, kc * RXP:(kc + 1) * RXP],
                              xpT[kc * P:(kc + 1) * P, :])
            nc.sync.dma_start(op_sb[:, kc * DM:(kc + 1) * DM],
                              opT[kc * P:(kc + 1) * P, :])
        dw_sb = wp.tile([RK, SH], F32R)
        nc.sync.dma_start(dw_sb[:], dwT[:].bitcast(F32R))
        cb_sb = wp.tile([P, 2], F32)
        db_sb = wp.tile([P, 2], F32)
        A_sb = wp.tile([P, 2 * DS], F32)
        D_sb = wp.tile([P, 2], F32)
        for t_, src in ((cb_sb, cb), (db_sb, dbc), (A_sb, Ac), (D_sb, Dc)):
            nc.sync.dma_start(t_[:], src[:])
        ident = wp.tile([P, P], BF16)
        nc.sync.dma_start(ident[:], idn[:])

        # long-lived activations
        u2 = [big.tile([P, B, L], BF16, name=f"u{c}") for c in range(2)]
        z2 = [big.tile([P, B, L], BF16, name=f"z{c}") for c in range(2)]
        d2 = [big.tile([P, B, L], F32, name=f"d{c}") for c in range(2)]
        yg2 = [big.tile([P, B, L], BF16, name=f"yg{c}") for c in range(2)]
        xd_sb = big.tile([RXP, BL], F32R)

        # ---- phase B: in-projections with conv folded into the u-matmuls ----
        with nc.named_scope("inproj"), \
                tc.tile_pool(name="wu4p", bufs=1) as wu4p, \
                tc.tile_pool(name="xk", bufs=2) as xkp, \
                tc.tile_pool(name="psB", bufs=4, space="PSUM") as psB:
            wu_sb = wu4p.tile([P, 4 * DC * SH], F32R)  # [:, k,(j,ch)]
            wz_sb = wu4p.tile([P, 4 * SH], F32R)
            for k in range(4):
                nc.sync.dma_start(wu_sb[:, k * DC * SH:(k + 1) * DC * SH],
                                  wu4[k * P:(k + 1) * P, :].bitcast(F32R))
                nc.sync.dma_start(wz_sb[:, k * SH:(k + 1) * SH],
                                  wzT[k * P:(k + 1) * P, :].bitcast(F32R))
            for b in range(B):
                for tt in range(4):
                    tok = b * L + tt * 512
                    xks = []
                    for k in range(4):
                        xk_t = xkp.tile([P, 515], F32R, name=f"xk{k}",
                                        tag=f"xk{k}")
                        if tt == 0:
                            nc.sync.dma_start(xk_t[:, 0:3],
                                              z3[:, 0:3].bitcast(F32R))
                            nc.sync.dma_start(
                                xk_t[:, 3:515],
                                xT[k * P:(k + 1) * P, tok:tok + 512]
                                .bitcast(F32R))
                        else:
                            nc.sync.dma_start(
                                xk_t[:, 0:515],
                                xT[k * P:(k + 1) * P, tok - 3:tok + 512]
                                .bitcast(F32R))
                        xks.append(xk_t)
                    for c in range(2):
                        ps_u = psB.tile([P, 512], F32, tag="psu")
                        first = True
                        for k in range(4):
                            for j in (3, 2, 1, 0):
                                w_sl = wu_sb[:, k * DC * SH + j * SH + c * P:
                                             k * DC * SH + j * SH + (c + 1) * P]
                                nc.tensor.matmul(
                                    ps_u[:], w_sl, xks[k][:, j:j + 512],
                                    start=first, stop=(k == 3 and j == 0))
                                first = False
                        nc.scalar.activation(u2[c][:, b, tt * 512:(tt + 1) * 512],
                                             ps_u[:], AF.Silu,
                                             bias=cb_sb[:, c:c + 1])
                        ps_z = psB.tile([P, 512], F32, tag="psz")
                        for k in range(4):
                            nc.tensor.matmul(
                                ps_z[:], wz_sb[:, k * SH + c * P:k * SH + (c + 1) * P],
                                xks[k][:, 3:515],
                                start=(k == 0), stop=(k == 3))
                        nc.scalar.activation(z2[c][:, b, tt * 512:(tt + 1) * 512],
                                             ps_z[:], AF.Silu)

        # ---- phase D: x_dbl partial -> per-batch AllReduce ----
        xd_reds = []
        with nc.named_scope("xdbl"), \
                tc.tile_pool(name="psD", bufs=2, space="PSUM") as psD, \
                tc.tile_pool(name="xds", bufs=2) as xds:
            for b in range(B):
                xd_in = dramp.tile([RXP, L], F32, name=f"xdi{b}")
                xd_red = dramp.tile([RXP, L], F32, name=f"xdr{b}")
                xd_reds.append(xd_red)
                for tt in range(4):
                    tok = b * L + tt * 512
                    ps = psD.tile([RXP, 512], F32, tag="psd")
                    for kc in range(2):
                        nc.tensor.matmul(
                            ps[:], xp_sb[:, kc * RXP:(kc + 1) * RXP],
                            u2[kc][:, b, tt * 512:(tt + 1) * 512],
                            start=(kc == 0), stop=(kc == 1))
                    xo = xds.tile([RXP, 512], F32, tag="xo")
                    nc.scalar.copy(xo[:], ps[:])
                    nc.sync.dma_start(xd_in[:, tt * 512:(tt + 1) * 512], xo[:])
                nc.gpsimd.collective_compute(
                    "AllReduce", ALU.add,
                    replica_groups=[[0, 1, 2, 3], [4, 5, 6, 7]],
                    ins=[xd_in.opt()], outs=[xd_red.opt()])
                bsl = slice(b * L, (b + 1) * L)
                nc.sync.dma_start(xd_sb[:, bsl], xd_red[:].bitcast(F32R))
                nc.sync.dma_start(lr_out[:, bsl], xd_sb[0:RK, bsl].bitcast(F32))

        # ---- phase E: delta = softplus(dt_lr @ dt_w.T + dt_b) ----
        with nc.named_scope("delta"), \
                tc.tile_pool(name="psE", bufs=2, space="PSUM") as psE, \
                tc.tile_pool(name="spt", bufs=2) as spt:
            for b in range(B):
                for c in range(2):
                    ps = psE.tile([P, L], F32, tag="pse")
                    for tt in range(4):
                        tok = b * L + tt * 512
                        nc.tensor.matmul(ps[:, tt * 512:(tt + 1) * 512],
                                         dw_sb[:, c * P:(c + 1) * P],
                                         xd_sb[0:RK, tok:tok + 512],
                                         start=True, stop=True)
                    ax = spt.tile([P, L], F32, tag="ax")
                    nc.scalar.activation(ax[:], ps[:], AF.Abs,
                                         bias=db_sb[:, c:c + 1])
                    ex = spt.tile([P, L], F32, tag="ex")
                    nc.scalar.activation(ex[:], ax[:], AF.Exp, scale=-1.0)
                    ln = spt.tile([P, L], F32, tag="ln")
                    nc.scalar.activation(ln[:], ex[:], AF.Ln, bias=1.0)
                    t1 = spt.tile([P, L], F32, tag="t1")
                    nc.vector.tensor_scalar_add(t1[:], ps[:],
                                                db_sb[:, c:c + 1])
                    nc.vector.scalar_tensor_tensor(
                        d2[c][:, b, :], t1[:], 0.0,
                        ln[:], op0=ALU.max, op1=ALU.add)
            for c in range(2):
                nc.sync.dma_start(d_out[c * P:(c + 1) * P, :],
                                  d2[c][:].rearrange("p b l -> p (b l)"))

        # ---- phase G: selective scan (full per-batch length, no carry) ----
        with nc.named_scope("scan"), \
                tc.tile_pool(name="psY", bufs=2, space="PSUM") as psY, \
                tc.tile_pool(name="sct", bufs=2) as sct:
            for b in range(B):
                bsl = slice(b * L, (b + 1) * L)
                dus = []
                for c in range(2):
                    du = sct.tile([P, L], BF16, name=f"du{c}", tag=f"du{c}",
                                  bufs=1)
                    nc.vector.tensor_mul(du[:], d2[c][:, b, :], u2[c][:, b, :])
                    dus.append(du)
                y_ps = [psY.tile([P, L], F32, name=f"yps{i}", tag="y")
                        for i in range(2)]
                for n in range(DS):
                    Bbc = sct.tile([P, L], BF16, tag="Bbc")
                    Cbc = sct.tile([P, L], BF16, tag="Cbc")
                    brow = xd_reds[b][32 + n:33 + n, :]
                    crow = xd_reds[b][64 + n:65 + n, :]
                    nc.gpsimd.dma_start(Bbc[:], bass.AP(
                        tensor=brow.tensor, offset=brow.offset,
                        ap=[[0, P]] + [list(p) for p in brow.ap[1:]]))
                    nc.gpsimd.dma_start(Cbc[:], bass.AP(
                        tensor=crow.tensor, offset=crow.offset,
                        ap=[[0, P]] + [list(p) for p in crow.ap[1:]]))
                    for c in range(2):
                        a_t = sct.tile([P, L], F32, tag="a")
                        nc.scalar.activation(
                            a_t[:], d2[c][:, b, :], AF.Exp,
                            scale=A_sb[:, c * DS + n:c * DS + n + 1])
                        b_t = sct.tile([P, L], BF16, tag="bt", bufs=3)
                        nc.vector.tensor_mul(b_t[:], dus[c][:], Bbc[:])
                        h_t = sct.tile([P, L], BF16, tag="h", bufs=3)
                        eng = nc.gpsimd if n in GP_NS else nc.vector
                        eng.tensor_tensor_scan(h_t[:], a_t[:], b_t[:], 0.0,
                                               op0=ALU.mult, op1=ALU.add)
                        hc_t = sct.tile([P, L], BF16, tag="hc", bufs=3)
                        nc.vector.tensor_mul(hc_t[:], h_t[:], Cbc[:])
                        for q in range(4):
                            hs = slice(q * 512, (q + 1) * 512)
                            nc.tensor.matmul(y_ps[c][:, hs], ident[:],
                                             hc_t[:, hs], start=(n == 0),
                                             stop=(n == DS - 1))
                for c in range(2):
                    y1 = sct.tile([P, L], F32, tag="a")
                    nc.vector.scalar_tensor_tensor(
                        y1[:], u2[c][:, b, :], D_sb[:, c:c + 1],
                        y_ps[c][:], op0=ALU.mult, op1=ALU.add)
                    nc.vector.tensor_mul(yg2[c][:, b, :], y1[:], z2[c][:, b, :])

        # ---- phase H: out-projection partials ----
        with nc.named_scope("outproj"), \
                tc.tile_pool(name="psH", bufs=2, space="PSUM") as psH, \
                tc.tile_pool(name="osb", bufs=2) as osb:
            for b in range(B):
                for tt in range(4):
                    tok = b * L + tt * 512
                    for dmt in range(4):
                        ps = psH.tile([P, 512], F32, tag="psh")
                        for kc in range(2):
                            nc.tensor.matmul(
                                ps[:],
                                op_sb[:, kc * DM + dmt * P:kc * DM + (dmt + 1) * P],
                                yg2[kc][:, b, tt * 512:(tt + 1) * 512],
                                start=(kc == 0), stop=(kc == 1))
                        o = osb.tile([P, 512], F32, tag="o")
                        nc.scalar.copy(o[:], ps[:])
                        nc.sync.dma_start(
                            y_out[dmt * P:(dmt + 1) * P, tok:tok + 512], o[:])

    nc.compile()
    return nc


def _get_nc():
    global _NC
    if _NC is None:
        _NC = _build()
    return _NC


def kernel(**inputs):
    x = np.asarray(inputs["x"], np.float32)
    nc = _get_nc()

    idn = np.eye(P, dtype=np.float32).astype(ml_dtypes.bfloat16)

    in_maps = []
    for g, pfx in enumerate(("f_", "r_")):
        W = np.asarray(inputs[pfx + "in_proj"], np.float32)
        conv_w = np.asarray(inputs[pfx + "conv_w"], np.float32)
        conv_b = np.asarray(inputs[pfx + "conv_b"], np.float32)
        x_proj = np.asarray(inputs[pfx + "x_proj"], np.float32)
        dt_w = np.asarray(inputs[pfx + "dt_w"], np.float32)
        dt_b = np.asarray(inputs[pfx + "dt_b"], np.float32)
        A = -np.exp(np.asarray(inputs[pfx + "A_log"], np.float32))
        D_ = np.asarray(inputs[pfx + "D"], np.float32)
        out_proj = np.asarray(inputs[pfx + "out_proj"], np.float32)

        xg = x if g == 0 else x[:, ::-1]
        xT = np.ascontiguousarray(xg.transpose(2, 0, 1).reshape(DM, BL))
        for s in range(4):
            ch = slice(s * SH, (s + 1) * SH)
            cm = lambda a: np.ascontiguousarray(a.astype(np.float32))
            # wu4[dm, j, ch] = W_u[ch, dm] * conv_w[ch, j]
            wu4 = (W[:DI][ch].T[:, None, :] *
                   conv_w[ch].T[None, :, :]).reshape(DM, DC * SH)
            xp_pad = np.zeros((SH, RXP), np.float32)
            xp_pad[:, 0:RK + DS] = x_proj[0:RK + DS, ch].T
            xp_pad[:, 64:80] = x_proj[RK + DS:RK + 2 * DS, ch].T
            in_maps.append({
                "xT": xT,
                "wu4": cm(wu4),
                "wzT": cm(W[DI:][ch].T),
                "cb": cm(conv_b[ch].reshape(2, P).T),
                "xpT": np.ascontiguousarray(xp_pad.astype(ml_dtypes.bfloat16)),
                "dwT": cm(dt_w[ch].T),
                "dbc": cm(dt_b[ch].reshape(2, P).T),
                "Ac": cm(A[ch].reshape(2, P, DS).transpose(1, 0, 2)
                         .reshape(P, 2 * DS)),
                "Dc": cm(D_[ch].reshape(2, P).T),
                "opT": np.ascontiguousarray(
                    out_proj[:, ch].T.astype(ml_dtypes.bfloat16)),
                "idn": idn,
                "z3": np.zeros((P, 4), np.float32),
            })

    global _LAST_IN_MAPS
    _LAST_IN_MAPS = in_maps
    res = run_bass_kernel_spmd(nc, in_maps, list(range(N_CORES))).results

    outs, dts, dts_lr = [], [], []
    for g in range(2):
        delta = np.concatenate([res[g * 4 + s]["d_out"] for s in range(4)],
                               axis=0)  # (DI, BL)
        dts.append(delta.reshape(DI, B, L).transpose(1, 2, 0))
        dts_lr.append(res[g * 4]["lr_out"].reshape(RK, B, L).transpose(1, 2, 0))
        y = np.sum([res[g * 4 + s]["y_out"] for s in range(4)], axis=0)
        y = y.reshape(DM, B, L).transpose(1, 2, 0)  # (B, L, DM)
        outs.append(y)
    outs[1] = outs[1][:, ::-1]
    out = (outs[0] + outs[1]) / 2
    return (out.astype(np.float32),
            np.stack(dts).astype(np.float32),
            np.stack(dts_lr).astype(np.float32))


# revision 19
# speedup vs baseline: 1.1314x; 1.0070x over previous
"""BiMamba (2-direction Mamba-1 SSM) Trainium2 kernel, 8 NeuronCores.

Sharding: direction (2) x d_inner shard (4 x 256 channels). Each core computes
its direction's projections for its 256 channels, the full selective scan for
those channels (tensor_tensor_scan over time, one lane per (channel, state)),
and a partial out-projection. x_dbl partials are AllReduced across each
direction's 4 cores; the host sums out-proj partials, concatenates delta
shards, flips the reverse direction, and averages directions.

v2: causal conv folded into the in-projection matmuls (per-tap scaled weight
copies accumulating into PSUM with shifted moving operands); B/C row
broadcasts via stride-0-partition DMA into SBUF bf16 (enables 2x-rate bf16
DVE multiplies and frees PE/PSUM); scans run the full per-batch length (no
carry chaining); gate/out-proj path in bf16.
"""
import numpy as np
from contextlib import ExitStack

import ml_dtypes
import concourse.bass as bass
import concourse.mybir as mybir
import concourse.tile as tile
from concourse import bacc
from concourse.bass_utils import run_bass_kernel_spmd

F32 = mybir.dt.float32
F32R = mybir.dt.float32r
BF16 = mybir.dt.bfloat16
AF = mybir.ActivationFunctionType
ALU = mybir.AluOpType

DM = 512        # d_model
DI = 1024       # d_inner
DS = 16         # d_state
DC = 4          # d_conv
RK = 32         # dt_rank
RXP = 80        # padded x_dbl rows: dt_lr 0:32, B 32:48, pad, C 64:80
B = 2
L = 2048
BL = B * L      # 4096 tokens, batch-major
SH = 256        # channels per core
P = 128
N_CORES = 8
GP_NS = ()  # gpsimd cannot run tensor_tensor_scan (engine check)

_NC = None
_LAST_IN_MAPS = None


def _build():
    nc = bacc.Bacc("TRN2", target_bir_lowering=False, debug=False,
                   num_devices=N_CORES)
    din = lambda n, s, d=F32: nc.declare_dram_parameter(n, list(s), d,
                                                        isOutput=False)
    dout = lambda n, s: nc.declare_dram_parameter(n, list(s), F32, isOutput=True)

    xT = din("xT", (DM, BL))
    wu4 = din("wu4", (DM, DC * SH))     # per-tap conv-scaled in_proj (u half)
    wzT = din("wzT", (DM, SH))
    cb = din("cb", (P, 2))
    xpT = din("xpT", (SH, RXP), BF16)
    dwT = din("dwT", (RK, SH))
    dbc = din("dbc", (P, 2))
    Ac = din("Ac", (P, 2 * DS))
    Dc = din("Dc", (P, 2))
    opT = din("opT", (SH, DM), BF16)
    idn = din("idn", (P, P), BF16)
    z3 = din("z3", (P, 4))

    d_out = dout("d_out", (SH, BL))
    lr_out = dout("lr_out", (RK, BL))
    y_out = dout("y_out", (DM, BL))

    with tile.TileContext(nc) as tc, ExitStack() as ctx:
        wp = ctx.enter_context(tc.tile_pool(name="weights", bufs=1))
        big = ctx.enter_context(tc.tile_pool(name="big", bufs=1))
        dramp = ctx.enter_context(tc.tile_pool(name="dram", bufs=1, space="DRAM"))

        xp_sb = wp.tile([P, 2 * RXP], BF16)
        op_sb = wp.tile([P, 2 * DM], BF16)
        for kc in range(2):
            nc.sync.dma_start(xp_sb[:, kc * RXP:(kc + 1) * RXP],
                              xpT[kc * P:(kc + 1) * P, :])
            nc.sync.dma_start(op_sb[:, kc * DM:(kc + 1) * DM],
                              opT[kc * P:(kc + 1) * P, :])
        dw_sb = wp.tile([RK, SH], F32R)
        nc.sync.dma_start(dw_sb[:], dwT[:].bitcast(F32R))
        cb_sb = wp.tile([P, 2], F32)
        db_sb = wp.tile([P, 2], F32)
        A_sb = wp.tile([P, 2 * DS], F32)
        D_sb = wp.tile([P, 2], F32)
        for t_, src in ((cb_sb, cb), (db_sb, dbc), (A_sb, Ac), (D_sb, Dc)):
            nc.sync.dma_start(t_[:], src[:])
        ident = wp.tile([P, P], BF16)
        nc.sync.dma_start(ident[:], idn[:])

        # long-lived activations
        u2 = [big.tile([P, B, L], BF16, name=f"u{c}") for c in range(2)]
        z2 = [big.tile([P, B, L], BF16, name=f"z{c}") for c in range(2)]
        d2 = [big.tile([P, B, L], F32, name=f"d{c}") for c in range(2)]
        yg2 = [big.tile([P, B, L], BF16, name=f"yg{c}") for c in range(2)]
        xd_sb = big.tile([RXP, BL], F32R)

        # ---- phase B: in-projections with conv folded into the u-matmuls ----
        with nc.named_scope("inproj"), \
                tc.tile_pool(name="wu4p", bufs=1) as wu4p, \
                tc.tile_pool(name="xk", bufs=2) as xkp, \
                tc.tile_pool(name="psB", bufs=4, space="PSUM") as psB:
            wu_sb = wu4p.tile([P, 4 * DC * SH], F32R)  # [:, k,(j,ch)]
            wz_sb = wu4p.tile([P, 4 * SH], F32R)
            for k in range(4):
                nc.sync.dma_start(wu_sb[:, k * DC * SH:(k + 1) * DC * SH],
                                  wu4[k * P:(k + 1) * P, :].bitcast(F32R))
                nc.sync.dma_start(wz_sb[:, k * SH:(k + 1) * SH],
                                  wzT[k * P:(k + 1) * P, :].bitcast(F32R))
            for b in range(B):
                for tt in range(4):
                    tok = b * L + tt * 512
                    xks = []
                    for k in range(4):
                        xk_t = xkp.tile([P, 515], F32R, name=f"xk{k}",
                                        tag=f"xk{k}")
                        if tt == 0:
                            nc.sync.dma_start(xk_t[:, 0:3],
                                              z3[:, 0:3].bitcast(F32R))
                            nc.sync.dma_start(
                                xk_t[:, 3:515],
                                xT[k * P:(k + 1) * P, tok:tok + 512]
                                .bitcast(F32R))
                        else:
                            nc.sync.dma_start(
                                xk_t[:, 0:515],
                                xT[k * P:(k + 1) * P, tok - 3:tok + 512]
                                .bitcast(F32R))
                        xks.append(xk_t)
                    for c in range(2):
                        ps_u = psB.tile([P, 512], F32, tag="psu")
                        first = True
                        for k in range(4):
                            for j in (3, 2, 1, 0):
                                w_sl = wu_sb[:, k * DC * SH + j * SH + c * P:
                                             k * DC * SH + j * SH + (c + 1) * P]
                                nc.tensor.matmul(
                                    ps_u[:], w_sl, xks[k][:, j:j + 512],
                                    start=first, stop=(k == 3 and j == 0))
                                first = False
                        nc.scalar.activation(u2[c][:, b, tt * 512:(tt + 1) * 512],
                                             ps_u[:], AF.Silu,
                                             bias=cb_sb[:, c:c + 1])
                        ps_z = psB.tile([P, 512], F32, tag="psz")
                        for k in range(4):
                            nc.tensor.matmul(
                                ps_z[:], wz_sb[:, k * SH + c * P:k * SH + (c + 1) * P],
                                xks[k][:, 3:515],
                                start=(k == 0), stop=(k == 3))
                        nc.scalar.activation(z2[c][:, b, tt * 512:(tt + 1) * 512],
                                             ps_z[:], AF.Silu)

        # ---- phase D: x_dbl partial -> per-batch AllReduce ----
        xd_reds = []
        with nc.named_scope("xdbl"), \
                tc.tile_pool(name="psD", bufs=2, space="PSUM") as psD, \
                tc.tile_pool(name="xds", bufs=2) as xds:
            for b in range(B):
                xd_in = dramp.tile([RXP, L], F32, name=f"xdi{b}")
                xd_red = dramp.tile([RXP, L], F32, name=f"xdr{b}")
                xd_reds.append(xd_red)
                for tt in range(4):
                    tok = b * L + tt * 512
                    ps = psD.tile([RXP, 512], F32, tag="psd")
                    for kc in range(2):
                        nc.tensor.matmul(
                            ps[:], xp_sb[:, kc * RXP:(kc + 1) * RXP],
                            u2[kc][:, b, tt * 512:(tt + 1) * 512],
                            start=(kc == 0), stop=(kc == 1))
                    xo = xds.tile([RXP, 512], F32, tag="xo")
                    nc.scalar.copy(xo[:], ps[:])
                    nc.sync.dma_start(xd_in[:, tt * 512:(tt + 1) * 512], xo[:])
                nc.gpsimd.collective_compute(
                    "AllReduce", ALU.add,
                    replica_groups=[[0, 1, 2, 3], [4, 5, 6, 7]],
                    ins=[xd_in.opt()], outs=[xd_red.opt()])
                bsl = slice(b * L, (b + 1) * L)
                nc.sync.dma_start(xd_sb[:, bsl], xd_red[:].bitcast(F32R))
                nc.sync.dma_start(lr_out[:, bsl], xd_sb[0:RK, bsl].bitcast(F32))

        # ---- phase E: delta = softplus(dt_lr @ dt_w.T + dt_b) ----
        with nc.named_scope("delta"), \
                tc.tile_pool(name="psE", bufs=2, space="PSUM") as psE, \
                tc.tile_pool(name="spt", bufs=2) as spt:
            for b in range(B):
                for c in range(2):
                    ps = psE.tile([P, L], F32, tag="pse")
                    for tt in range(4):
                        tok = b * L + tt * 512
                        nc.tensor.matmul(ps[:, tt * 512:(tt + 1) * 512],
                                         dw_sb[:, c * P:(c + 1) * P],
                                         xd_sb[0:RK, tok:tok + 512],
                                         start=True, stop=True)
                    ax = spt.tile([P, L], F32, tag="ax")
                    nc.scalar.activation(ax[:], ps[:], AF.Abs,
                                         bias=db_sb[:, c:c + 1])
                    ex = spt.tile([P, L], F32, tag="ex")
                    nc.scalar.activation(ex[:], ax[:], AF.Exp, scale=-1.0)
                    ln = spt.tile([P, L], F32, tag="ln")
                    nc.scalar.activation(ln[:], ex[:], AF.Ln, bias=1.0)
                    t1 = spt.tile([P, L], F32, tag="t1")
                    nc.vector.tensor_scalar_add(t1[:], ps[:],
                                                db_sb[:, c:c + 1])
                    nc.vector.scalar_tensor_tensor(
                        d2[c][:, b, :], t1[:], 0.0,
                        ln[:], op0=ALU.max, op1=ALU.add)
            for c in range(2):
                nc.sync.dma_start(d_out[c * P:(c + 1) * P, :],
                                  d2[c][:].rearrange("p b l -> p (b l)"))

        # ---- phase G: selective scan (full per-batch length, no carry) ----
        with nc.named_scope("scan"), \
                tc.tile_pool(name="psY", bufs=2, space="PSUM") as psY, \
                tc.tile_pool(name="sct", bufs=2) as sct:
            for b in range(B):
                bsl = slice(b * L, (b + 1) * L)
                dus = []
                for c in range(2):
                    du = sct.tile([P, L], BF16, name=f"du{c}", tag=f"du{c}",
                                  bufs=1)
                    nc.vector.tensor_mul(du[:], d2[c][:, b, :], u2[c][:, b, :])
                    dus.append(du)
                y_ps = [psY.tile([P, L], F32, name=f"yps{i}", tag="y")
                        for i in range(2)]
                for n in range(DS):
                    Bbc = sct.tile([P, L], BF16, tag="Bbc")
                    Cbc = sct.tile([P, L], BF16, tag="Cbc")
                    brow = xd_reds[b][32 + n:33 + n, :]
                    crow = xd_reds[b][64 + n:65 + n, :]
                    nc.gpsimd.dma_start(Bbc[:], bass.AP(
                        tensor=brow.tensor, offset=brow.offset,
                        ap=[[0, P]] + [list(p) for p in brow.ap[1:]]))
                    nc.gpsimd.dma_start(Cbc[:], bass.AP(
                        tensor=crow.tensor, offset=crow.offset,
                        ap=[[0, P]] + [list(p) for p in crow.ap[1:]]))
                    for c in range(2):
                        a_t = sct.tile([P, L], BF16, tag="a", bufs=4)
                        nc.scalar.activation(
                            a_t[:], d2[c][:, b, :], AF.Exp,
                            scale=A_sb[:, c * DS + n:c * DS + n + 1])
                        b_t = sct.tile([P, L], BF16, tag="bt", bufs=3)
                        nc.vector.tensor_mul(b_t[:], dus[c][:], Bbc[:])
                        h_t = sct.tile([P, L], BF16, tag="h", bufs=3)
                        eng = nc.gpsimd if n in GP_NS else nc.vector
                        eng.tensor_tensor_scan(h_t[:], a_t[:], b_t[:], 0.0,
                                               op0=ALU.mult, op1=ALU.add)
                        hc_t = sct.tile([P, L], BF16, tag="hc", bufs=3)
                        nc.vector.tensor_mul(hc_t[:], h_t[:], Cbc[:])
                        for q in range(4):
                            hs = slice(q * 512, (q + 1) * 512)
                            nc.tensor.matmul(y_ps[c][:, hs], ident[:],
                                             hc_t[:, hs], start=(n == 0),
                                             stop=(n == DS - 1))
                for c in range(2):
                    y1 = sct.tile([P, L], F32, tag="y1", bufs=1)
                    nc.vector.scalar_tensor_tensor(
                        y1[:], u2[c][:, b, :], D_sb[:, c:c + 1],
                        y_ps[c][:], op0=ALU.mult, op1=ALU.add)
                    nc.vector.tensor_mul(yg2[c][:, b, :], y1[:], z2[c][:, b, :])

        # ---- phase H: out-projection partials ----
        with nc.named_scope("outproj"), \
                tc.tile_pool(name="psH", bufs=2, space="PSUM") as psH, \
                tc.tile_pool(name="osb", bufs=2) as osb:
            for b in range(B):
                for tt in range(4):
                    tok = b * L + tt * 512
                    for dmt in range(4):
                        ps = psH.tile([P, 512], F32, tag="psh")
                        for kc in range(2):
                            nc.tensor.matmul(
                                ps[:],
                                op_sb[:, kc * DM + dmt * P:kc * DM + (dmt + 1) * P],
                                yg2[kc][:, b, tt * 512:(tt + 1) * 512],
                                start=(kc == 0), stop=(kc == 1))
                        o = osb.tile([P, 512], F32, tag="o")
                        nc.scalar.copy(o[:], ps[:])
                        nc.sync.dma_start(
                            y_out[dmt * P:(dmt + 1) * P, tok:tok + 512], o[:])

    nc.compile()
    return nc


def _get_nc():
    global _NC
    if _NC is None:
        _NC = _build()
    return _NC


def kernel(**inputs):
    x = np.asarray(inputs["x"], np.float32)
    nc = _get_nc()

    idn = np.eye(P, dtype=np.float32).astype(ml_dtypes.bfloat16)

    in_maps = []
    for g, pfx in enumerate(("f_", "r_")):
        W = np.asarray(inputs[pfx + "in_proj"], np.float32)
        conv_w = np.asarray(inputs[pfx + "conv_w"], np.float32)
        conv_b = np.asarray(inputs[pfx + "conv_b"], np.float32)
        x_proj = np.asarray(inputs[pfx + "x_proj"], np.float32)
        dt_w = np.asarray(inputs[pfx + "dt_w"], np.float32)
        dt_b = np.asarray(inputs[pfx + "dt_b"], np.float32)
        A = -np.exp(np.asarray(inputs[pfx + "A_log"], np.float32))
        D_ = np.asarray(inputs[pfx + "D"], np.float32)
        out_proj = np.asarray(inputs[pfx + "out_proj"], np.float32)

        xg = x if g == 0 else x[:, ::-1]
        xT = np.ascontiguousarray(xg.transpose(2, 0, 1).reshape(DM, BL))
        for s in range(4):
            ch = slice(s * SH, (s + 1) * SH)
            cm = lambda a: np.ascontiguousarray(a.astype(np.float32))
            # wu4[dm, j, ch] = W_u[ch, dm] * conv_w[ch, j]
            wu4 = (W[:DI][ch].T[:, None, :] *
                   conv_w[ch].T[None, :, :]).reshape(DM, DC * SH)
            xp_pad = np.zeros((SH, RXP), np.float32)
            xp_pad[:, 0:RK + DS] = x_proj[0:RK + DS, ch].T
            xp_pad[:, 64:80] = x_proj[RK + DS:RK + 2 * DS, ch].T
            in_maps.append({
                "xT": xT,
                "wu4": cm(wu4),
                "wzT": cm(W[DI:][ch].T),
                "cb": cm(conv_b[ch].reshape(2, P).T),
                "xpT": np.ascontiguousarray(xp_pad.astype(ml_dtypes.bfloat16)),
                "dwT": cm(dt_w[ch].T),
                "dbc": cm(dt_b[ch].reshape(2, P).T),
                "Ac": cm(A[ch].reshape(2, P, DS).transpose(1, 0, 2)
                         .reshape(P, 2 * DS)),
                "Dc": cm(D_[ch].reshape(2, P).T),
                "opT": np.ascontiguousarray(
                    out_proj[:, ch].T.astype(ml_dtypes.bfloat16)),
                "idn": idn,
                "z3": np.zeros((P, 4), np.float32),
            })

    global _LAST_IN_MAPS
    _LAST_IN_MAPS = in_maps
    res = run_bass_kernel_spmd(nc, in_maps, list(range(N_CORES))).results

    outs, dts, dts_lr = [], [], []
    for g in range(2):
        delta = np.concatenate([res[g * 4 + s]["d_out"] for s in range(4)],
                               axis=0)  # (DI, BL)
        dts.append(delta.reshape(DI, B, L).transpose(1, 2, 0))
        dts_lr.append(res[g * 4]["lr_out"].reshape(RK, B, L).transpose(1, 2, 0))
        y = np.sum([res[g * 4 + s]["y_out"] for s in range(4)], axis=0)
        y = y.reshape(DM, B, L).transpose(1, 2, 0)  # (B, L, DM)
        outs.append(y)
    outs[1] = outs[1][:, ::-1]
    out = (outs[0] + outs[1]) / 2
    return (out.astype(np.float32),
            np.stack(dts).astype(np.float32),
            np.stack(dts_lr).astype(np.float32))
